# revision 13
# baseline (speedup 1.0000x reference)
"""Neural CDE Euler scan on 8 TRN2 NeuronCores.

Strategy
--------
Data-parallel: batch 512 is split 64-per-core; each core runs the full
200-step Euler scan with everything SBUF-resident (zero per-step HBM
traffic, no collectives).

Per step, per core (64 batch):
  uT   = transpose(u)                      # PE,  [64,128] -> [128,64]
  h1b  = softplus(uT.T @ W0T)              # PE (stationary=uT, stream W0T) -> [64,512], ACT
  h1f  = transpose(h1b)                    # PE x4 -> [128, 4*64]
  h2b  = softplus(h1f.T @ W1T)             # PE (stationary=h1f chunks, stream W1T) k-accum
  h2f  = transpose(h2b)                    # PE x4
  z    = h2f.T @ W2T                       # PE -> psum [64, 2048]
  g    = tanh(z)                           # ACT
  ein  = g * dX[s] (broadcast over h)      # DVE
  uadd = sum_d ein                         # DVE -> [64,128]
  u    = (u + DT*lorenz96(u)) + uadd       # DVE (free-dim rolls via extended buffer)

Matmul moving-operand dtype is switchable: float32 (4 cyc/col, exact) or
float32r (1 cyc/col at N>=256).  Biases are all zero in this problem; a
guard asserts that so no bias path is emitted.
"""

import os
import sys
import numpy as np

sys.path.insert(0, "/opt/trn_rl_repo")

B, H, D, W, N = 512, 128, 16, 512, 201
DT = np.float32(0.01)
STEPS = 200
F_LORENZ = np.float32(8.0)
NCORES = 8
BL = B // NCORES  # 64 batch per core

# "f32"  : exact fp32 matmuls (4 cyc/col streams)
# "f32r" : float32r streams (1 cyc/col at N>=256)
STREAM_MODE = os.environ.get("NCDE_STREAM", "f32")
TRACE = bool(int(os.environ.get("NCDE_TRACE", "0")))

LAST_RESULTS = {}  # stash for test.py introspection


def _compute_dX(ts, coeff_a, coeff_b, coeff_c, coeff_d):
    """Bit-exact fp32 replication of the reference interpolation increments."""
    n = np.arange(STEPS, dtype=np.float32)
    t0 = (ts[0] + n * DT).astype(np.float32)
    t1 = (t0 + DT).astype(np.float32)

    def interp(t):
        idx = np.clip(np.searchsorted(ts, t, side="right") - 1, 0, N - 2)
        frac = (t - ts[idx]).astype(np.float32)
        f = frac[None, :, None]
        a = coeff_a[:, idx]
        b = coeff_b[:, idx]
        c = coeff_c[:, idx]
        d = coeff_d[:, idx]
        return (a + f * (b + f * (c + f * d))).astype(np.float32)

    return (interp(t1) - interp(t0)).astype(np.float32)  # [B, STEPS, D]


def _build(stream_mode, steps=STEPS):
    import concourse.mybir as mybir
    import concourse.tile as tile
    from concourse import bacc

    F32 = mybir.dt.float32
    F32R = mybir.dt.float32r
    split = stream_mode == "f32rs"
    sdt = F32R if split else F32

    def scast(ap):
        return ap

    nc = bacc.Bacc()

    # DRAM I/O
    u0_d = nc.dram_tensor("u0c", [BL, H], F32, kind="ExternalInput")
    w0t_d = nc.dram_tensor("w0t", [H, W], F32, kind="ExternalInput")         # W0.T
    w1t_d = nc.dram_tensor("w1t", [W, W], F32, kind="ExternalInput")         # W1.T
    w2t_d = nc.dram_tensor("w2t", [W, H * D], F32, kind="ExternalInput")     # W2.T
    dx_d = nc.dram_tensor("dxc", [BL, STEPS * D], F32, kind="ExternalInput")
    id_d = nc.dram_tensor("id64", [BL, BL], F32, kind="ExternalInput")
    out_d = nc.dram_tensor("uout", [BL, H], F32, kind="ExternalOutput")

    KC = W // 128  # 4 k-chunks for W1/W2 contraction

    with tile.TileContext(nc) as tc:
        with tc.tile_pool(name="per", bufs=1) as per, \
             tc.tile_pool(name="act", bufs=2) as actp, \
             tc.tile_pool(name="ps_s", bufs=2, space="PSUM") as ps_s, \
             tc.tile_pool(name="ps_t", bufs=2, space="PSUM") as ps_t, \
             tc.tile_pool(name="ps_l2", bufs=1, space="PSUM") as ps_l2:
            # PSUM budget (8 banks): ps_s 1 tag x2 bufs = 2, ps_t 1 tag x2 = 2, ps_l2 = 4

            # ---- persistent tiles ----
            # fp32(/f32r) matmuls tolerate only ONE semaphore wait (LDW+MM
            # pair), so every tile the PE reads is produced by a DVE copy
            # (single feeder engine); DMAs land in staging tiles only.
            w0t = per.tile([H, W], sdt, tag="w0t")                  # [128, 512]
            w1t = per.tile([128, KC * W], sdt, tag="w1t")           # k-chunk major
            w2t = per.tile([128, KC * H * D], sdt, tag="w2t")       # k-chunk major
            dxs = per.tile([BL, STEPS * D], F32, tag="dxs")
            id64 = per.tile([BL, BL], F32, tag="id64")
            # state, extended for circular rolls: [u126 u127 | u0..u127 | u0 u1]
            uext = per.tile([BL, H + 4], F32, tag="uext")

            with tc.tile_pool(name="stage", bufs=1) as stage:
                w0s = stage.tile([H, W], F32, tag="w0s")
                w1s = stage.tile([128, KC * W], F32, tag="w1s")
                w2s = stage.tile([128, KC * H * D], F32, tag="w2s")
                ids = stage.tile([BL, BL], F32, tag="ids")
                u0s = stage.tile([BL, H], F32, tag="u0s")
                nc.sync.dma_start(out=w0s[:], in_=w0t_d[:])
                for k in range(KC):
                    nc.sync.dma_start(out=w1s[:, k * W:(k + 1) * W],
                                      in_=w1t_d[k * 128:(k + 1) * 128, :])
                    nc.sync.dma_start(out=w2s[:, k * H * D:(k + 1) * H * D],
                                      in_=w2t_d[k * 128:(k + 1) * 128, :])
                nc.sync.dma_start(out=ids[:], in_=id_d[:])
                nc.sync.dma_start(out=u0s[:], in_=u0_d[:])
                nc.sync.dma_start(out=dxs[:], in_=dx_d[:])
                nc.vector.tensor_copy(out=w0t[:], in_=w0s[:])
                nc.vector.tensor_copy(out=w1t[:], in_=w1s[:])
                nc.vector.tensor_copy(out=w2t[:], in_=w2s[:])
                nc.vector.tensor_copy(out=id64[:], in_=ids[:])
                nc.vector.tensor_copy(out=uext[:, 2:2 + H], in_=u0s[:])
                nc.vector.tensor_copy(out=uext[:, 0:2], in_=uext[:, H:H + 2])
                nc.vector.tensor_copy(out=uext[:, H + 2:H + 4], in_=uext[:, 2:4])

            u_ap = uext[:, 2:2 + H]         # u
            um2_ap = uext[:, 0:H]           # roll(u, 2)  : u[i-2]
            um1_ap = uext[:, 1:1 + H]       # roll(u, 1)  : u[i-1]
            up1_ap = uext[:, 3:3 + H]       # roll(u, -1) : u[i+1]

            ND = H * D  # 2048

            for s in range(steps):
                # uT = u.T  (PE transpose [64,128] -> [128,64])
                tp_u = ps_t.tile([128, BL], F32, tag="tp")
                nc.tensor.transpose(tp_u[:], u_ap, id64[:])
                uT = actp.tile([128, BL], F32, tag="uT")
                nc.vector.tensor_copy(out=uT[:], in_=tp_u[:])

                # L0: h1b[64, 512] = uT.T @ W0T
                p0 = ps_s.tile([BL, W], F32, tag="pp")
                nc.tensor.matmul(p0[:], lhsT=scast(uT[:]), rhs=scast(w0t[:]),
                                 start=True, stop=True)
                # softplus(x) = ln(exp(x) + 1) — Softplus has no PWP table in
                # this compiler build; Exp and Ln share one table set.
                e1 = actp.tile([BL, W], F32, tag="e1")
                nc.scalar.activation(e1[:], p0[:],
                                     func=mybir.ActivationFunctionType.Exp)
                h1b = actp.tile([BL, W], F32, tag="h1b")
                nc.scalar.activation(h1b[:], e1[:],
                                     func=mybir.ActivationFunctionType.Ln,
                                     bias=1.0)

                # h1f chunks [128, 4*64]
                tp1 = ps_t.tile([128, KC * BL], F32, tag="tp")
                for k in range(KC):
                    nc.tensor.transpose(tp1[:, k * BL:(k + 1) * BL],
                                        h1b[:, k * 128:(k + 1) * 128], id64[:])
                h1f = actp.tile([128, KC * BL], F32, tag="h1f")
                nc.vector.tensor_copy(out=h1f[:], in_=tp1[:])

                # L1: h2b[64, 512] = sum_k h1f_k.T @ W1T_k
                p1 = ps_s.tile([BL, W], F32, tag="pp")
                for k in range(KC):
                    nc.tensor.matmul(p1[:],
                                     lhsT=scast(h1f[:, k * BL:(k + 1) * BL]),
                                     rhs=scast(w1t[:, k * W:(k + 1) * W]),
                                     start=(k == 0), stop=(k == KC - 1))
                e2 = actp.tile([BL, W], F32, tag="e2")
                nc.scalar.activation(e2[:], p1[:],
                                     func=mybir.ActivationFunctionType.Exp)
                h2b = actp.tile([BL, W], F32, tag="h2b")
                nc.scalar.activation(h2b[:], e2[:],
                                     func=mybir.ActivationFunctionType.Ln,
                                     bias=1.0)

                # h2f chunks
                tp2 = ps_t.tile([128, KC * BL], F32, tag="tp")
                for k in range(KC):
                    nc.tensor.transpose(tp2[:, k * BL:(k + 1) * BL],
                                        h2b[:, k * 128:(k + 1) * 128], id64[:])
                h2f = actp.tile([128, KC * BL], F32, tag="h2f")
                nc.vector.tensor_copy(out=h2f[:], in_=tp2[:])

                # L2: z[64, 2048] = sum_k h2f_k.T @ W2T_k ; n-chunks of 512
                pz = ps_l2.tile([BL, ND], F32, tag="pz")
                g = actp.tile([BL, ND], F32, tag="g")
                NcN = ND // 512  # 4
                for n in range(NcN):
                    for k in range(KC):
                        nc.tensor.matmul(
                            pz[:, n * 512:(n + 1) * 512],
                            lhsT=scast(h2f[:, k * BL:(k + 1) * BL]),
                            rhs=scast(w2t[:, k * ND + n * 512:k * ND + (n + 1) * 512]),
                            start=(k == 0), stop=(k == KC - 1))
                    nc.scalar.activation(g[:, n * 512:(n + 1) * 512],
                                         pz[:, n * 512:(n + 1) * 512],
                                         func=mybir.ActivationFunctionType.Tanh)

                # einsum: uadd[b,h] = sum_d g[b, h*16+d] * dX[b, s*16+d]
                ein = actp.tile([BL, ND], F32, tag="ein")
                dx_bc = dxs[:, s * D:(s + 1) * D].unsqueeze(1).broadcast_to([BL, H, D])
                nc.vector.tensor_tensor(
                    out=ein[:].rearrange("b (h d) -> b h d", d=D),
                    in0=g[:].rearrange("b (h d) -> b h d", d=D),
                    in1=dx_bc, op=mybir.AluOpType.mult)
                uadd = actp.tile([BL, H], F32, tag="uadd")
                nc.vector.tensor_reduce(
                    out=uadd[:], in_=ein[:].rearrange("b (h d) -> b h d", d=D),
                    axis=mybir.AxisListType.X, op=mybir.AluOpType.add)

                # lorenz drift + state update (replicates reference association)
                t1 = actp.tile([BL, H], F32, tag="t1")
                nc.vector.tensor_tensor(out=t1[:], in0=up1_ap, in1=um2_ap,
                                        op=mybir.AluOpType.subtract)
                t2 = actp.tile([BL, H], F32, tag="t2")
                nc.vector.tensor_tensor(out=t2[:], in0=t1[:], in1=um1_ap,
                                        op=mybir.AluOpType.mult)
                # lor = t2 - u + F ; u = (u + lor*DT) + uadd
                t3 = actp.tile([BL, H], F32, tag="t3")
                nc.vector.scalar_tensor_tensor(out=t3[:], in0=u_ap, scalar=-1.0,
                                               in1=t2[:], op0=mybir.AluOpType.mult,
                                               op1=mybir.AluOpType.add)  # t2 - u
                t4 = actp.tile([BL, H], F32, tag="t4")
                nc.vector.tensor_scalar(out=t4[:], in0=t3[:],
                                        scalar1=float(F_LORENZ), scalar2=float(DT),
                                        op0=mybir.AluOpType.add,
                                        op1=mybir.AluOpType.mult)  # (lor)*DT
                t5 = actp.tile([BL, H], F32, tag="t5")
                nc.vector.tensor_tensor(out=t5[:], in0=u_ap, in1=t4[:],
                                        op=mybir.AluOpType.add)  # u + lor*DT
                nc.vector.tensor_tensor(out=u_ap, in0=t5[:], in1=uadd[:],
                                        op=mybir.AluOpType.add)
                # refresh wrap columns
                nc.vector.tensor_copy(out=uext[:, 0:2], in_=uext[:, H:H + 2])
                nc.vector.tensor_copy(out=uext[:, H + 2:H + 4], in_=uext[:, 2:4])

            nc.sync.dma_start(out=out_d[:], in_=u_ap)

    nc.compile()
    return nc


_BUILT = {}


def _get_nc(stream_mode):
    if stream_mode not in _BUILT:
        _BUILT[stream_mode] = _build(stream_mode)
    return _BUILT[stream_mode]


def kernel(u0, ts, coeff_a, coeff_b, coeff_c, coeff_d, W0, b0, W1, b1, W2, b2):
    from concourse.bass_utils import run_bass_kernel_spmd

    u0 = np.ascontiguousarray(np.asarray(u0, np.float32))
    ts = np.asarray(ts, np.float32)
    coeff_a = np.asarray(coeff_a, np.float32)
    coeff_b = np.asarray(coeff_b, np.float32)
    coeff_c = np.asarray(coeff_c, np.float32)
    coeff_d = np.asarray(coeff_d, np.float32)
    assert not (np.any(b0) or np.any(b1) or np.any(b2)), \
        "nonzero MLP biases not supported by this kernel build"

    dX = _compute_dX(ts, coeff_a, coeff_b, coeff_c, coeff_d)  # [B, S, D]
    dXf = np.ascontiguousarray(dX.reshape(B, STEPS * D))

    w0t = np.ascontiguousarray(np.asarray(W0, np.float32).T)   # [128, 512]
    w1t = np.ascontiguousarray(np.asarray(W1, np.float32).T)   # [512, 512]
    w2t = np.ascontiguousarray(np.asarray(W2, np.float32).T)   # [512, 2048]
    ident = np.eye(BL, dtype=np.float32)

    nc = _get_nc(STREAM_MODE)
    in_maps = []
    for c in range(NCORES):
        sl = slice(c * BL, (c + 1) * BL)
        in_maps.append(dict(
            u0c=u0[sl], w0t=w0t, w1t=w1t, w2t=w2t,
            dxc=dXf[sl], id64=ident,
        ))

    br = run_bass_kernel_spmd(nc, in_maps, list(range(NCORES)), trace=TRACE)
    LAST_RESULTS["bench"] = br

    out = np.empty((B, H), dtype=np.float32)
    for c in range(NCORES):
        out[c * BL:(c + 1) * BL] = br.results[c]["uout"]
    return out


# revision 17
# speedup vs baseline: 11.5841x; 11.5841x over previous
"""Neural CDE Euler scan on 8 TRN2 NeuronCores.

Strategy
--------
Data-parallel: batch 512 is split 64-per-core; each core runs the full
200-step Euler scan with everything SBUF-resident (zero per-step HBM
traffic, no collectives).

Per step, per core (64 batch):
  uT   = transpose(u)                      # PE,  [64,128] -> [128,64]
  h1b  = softplus(uT.T @ W0T)              # PE (stationary=uT, stream W0T) -> [64,512], ACT
  h1f  = transpose(h1b)                    # PE x4 -> [128, 4*64]
  h2b  = softplus(h1f.T @ W1T)             # PE (stationary=h1f chunks, stream W1T) k-accum
  h2f  = transpose(h2b)                    # PE x4
  z    = h2f.T @ W2T                       # PE -> psum [64, 2048]
  g    = tanh(z)                           # ACT
  ein  = g * dX[s] (broadcast over h)      # DVE
  uadd = sum_d ein                         # DVE -> [64,128]
  u    = (u + DT*lorenz96(u)) + uadd       # DVE (free-dim rolls via extended buffer)

Matmul moving-operand dtype is switchable: float32 (4 cyc/col, exact) or
float32r (1 cyc/col at N>=256).  Biases are all zero in this problem; a
guard asserts that so no bias path is emitted.
"""

import os
import sys
import numpy as np

sys.path.insert(0, "/opt/trn_rl_repo")

B, H, D, W, N = 512, 128, 16, 512, 201
DT = np.float32(0.01)
STEPS = 200
F_LORENZ = np.float32(8.0)
NCORES = 8
BL = B // NCORES  # 64 batch per core

# "f32"  : exact fp32 matmuls (4 cyc/col streams)
# "f32r" : float32r streams (1 cyc/col at N>=256)
STREAM_MODE = os.environ.get("NCDE_STREAM", "f32")
TRACE = bool(int(os.environ.get("NCDE_TRACE", "0")))

LAST_RESULTS = {}  # stash for test.py introspection


def _compute_dX(ts, coeff_a, coeff_b, coeff_c, coeff_d):
    """Bit-exact fp32 replication of the reference interpolation increments."""
    n = np.arange(STEPS, dtype=np.float32)
    t0 = (ts[0] + n * DT).astype(np.float32)
    t1 = (t0 + DT).astype(np.float32)

    def interp(t):
        idx = np.clip(np.searchsorted(ts, t, side="right") - 1, 0, N - 2)
        frac = (t - ts[idx]).astype(np.float32)
        f = frac[None, :, None]
        a = coeff_a[:, idx]
        b = coeff_b[:, idx]
        c = coeff_c[:, idx]
        d = coeff_d[:, idx]
        return (a + f * (b + f * (c + f * d))).astype(np.float32)

    return (interp(t1) - interp(t0)).astype(np.float32)  # [B, STEPS, D]


def _build(stream_mode, steps=STEPS):
    import concourse.mybir as mybir
    import concourse.tile as tile
    from concourse import bacc

    F32 = mybir.dt.float32
    F32R = mybir.dt.float32r
    split = stream_mode == "f32rs"
    sdt = F32R if split else F32

    def scast(ap):
        return ap

    nc = bacc.Bacc()

    # DRAM I/O
    u0_d = nc.dram_tensor("u0c", [BL, H], F32, kind="ExternalInput")
    w0t_d = nc.dram_tensor("w0t", [H, W], F32, kind="ExternalInput")         # W0.T
    w1t_d = nc.dram_tensor("w1t", [W, W], F32, kind="ExternalInput")         # W1.T
    w2t_d = nc.dram_tensor("w2t", [W, H * D], F32, kind="ExternalInput")     # W2.T
    dx_d = nc.dram_tensor("dxc", [BL, STEPS * D], F32, kind="ExternalInput")
    id_d = nc.dram_tensor("id64", [BL, BL], F32, kind="ExternalInput")
    out_d = nc.dram_tensor("uout", [BL, H], F32, kind="ExternalOutput")

    KC = W // 128  # 4 k-chunks for W1/W2 contraction

    with tile.TileContext(nc) as tc:
        with tc.tile_pool(name="per", bufs=1) as per, \
             tc.tile_pool(name="act", bufs=2) as actp, \
             tc.tile_pool(name="ps_s", bufs=2, space="PSUM") as ps_s, \
             tc.tile_pool(name="ps_t", bufs=2, space="PSUM") as ps_t, \
             tc.tile_pool(name="ps_l2", bufs=1, space="PSUM") as ps_l2:
            # PSUM budget (8 banks): ps_s 1 tag x2 bufs = 2, ps_t 1 tag x2 = 2, ps_l2 = 4

            # ---- persistent tiles ----
            # fp32(/f32r) matmuls tolerate only ONE semaphore wait (LDW+MM
            # pair), so every tile the PE reads is produced by a DVE copy
            # (single feeder engine); DMAs land in staging tiles only.
            dxs = per.tile([BL, STEPS * D], F32, tag="dxs")
            id64 = per.tile([BL, BL], F32, tag="id64")
            # state, extended for circular rolls: [u126 u127 | u0..u127 | u0 u1]
            uext = per.tile([BL, H + 4], F32, tag="uext")
            if split:
                w0h = per.tile([H, W], F32R, tag="w0h")
                w0l = per.tile([H, W], F32R, tag="w0l")
                w1h = per.tile([128, KC * W], F32R, tag="w1h")
                w1l = per.tile([128, KC * W], F32R, tag="w1l")
                w2h = per.tile([128, KC * H * D], F32R, tag="w2h")
                w2l = per.tile([128, KC * H * D], F32R, tag="w2l")
            else:
                w0t = per.tile([H, W], F32, tag="w0t")              # [128, 512]
                w1t = per.tile([128, KC * W], F32, tag="w1t")       # k-chunk major
                w2t = per.tile([128, KC * H * D], F32, tag="w2t")   # k-chunk major

            with tc.tile_pool(name="stage", bufs=1) as stage:
                w0s = stage.tile([H, W], F32, tag="w0s")
                w1s = stage.tile([128, KC * W], F32, tag="w1s")
                w2s = stage.tile([128, KC * H * D], F32, tag="w2s")
                ids = stage.tile([BL, BL], F32, tag="ids")
                u0s = stage.tile([BL, H], F32, tag="u0s")
                nc.sync.dma_start(out=w0s[:], in_=w0t_d[:])
                for k in range(KC):
                    nc.sync.dma_start(out=w1s[:, k * W:(k + 1) * W],
                                      in_=w1t_d[k * 128:(k + 1) * 128, :])
                    nc.sync.dma_start(out=w2s[:, k * H * D:(k + 1) * H * D],
                                      in_=w2t_d[k * 128:(k + 1) * 128, :])
                nc.sync.dma_start(out=ids[:], in_=id_d[:])
                nc.sync.dma_start(out=u0s[:], in_=u0_d[:])
                nc.sync.dma_start(out=dxs[:], in_=dx_d[:])
                if split:
                    # hi = round_f32r(w); lo = round_f32r(w - hi)
                    for ws, wh, wl in ((w0s, w0h, w0l), (w1s, w1h, w1l),
                                       (w2s, w2h, w2l)):
                        nc.vector.tensor_copy(out=wh[:], in_=ws[:])
                        nc.vector.tensor_tensor(out=wl[:], in0=ws[:], in1=wh[:],
                                                op=mybir.AluOpType.subtract)
                else:
                    nc.vector.tensor_copy(out=w0t[:], in_=w0s[:])
                    nc.vector.tensor_copy(out=w1t[:], in_=w1s[:])
                    nc.vector.tensor_copy(out=w2t[:], in_=w2s[:])
                nc.vector.tensor_copy(out=id64[:], in_=ids[:])
                nc.vector.tensor_copy(out=uext[:, 2:2 + H], in_=u0s[:])
                nc.vector.tensor_copy(out=uext[:, 0:2], in_=uext[:, H:H + 2])
                nc.vector.tensor_copy(out=uext[:, H + 2:H + 4], in_=uext[:, 2:4])

            u_ap = uext[:, 2:2 + H]         # u
            um2_ap = uext[:, 0:H]           # roll(u, 2)  : u[i-2]
            um1_ap = uext[:, 1:1 + H]       # roll(u, 1)  : u[i-1]
            up1_ap = uext[:, 3:3 + H]       # roll(u, -1) : u[i+1]

            ND = H * D  # 2048

            NcN = ND // 512  # 4
            for s in range(steps):
                # uT = u.T  (PE transpose [64,128] -> [128,64])
                tp_u = ps_t.tile([128, BL], F32, tag="tp")
                nc.tensor.transpose(tp_u[:], u_ap, id64[:])
                if split:
                    # pack [hi | lo] as stationary columns
                    uhl = actp.tile([128, 2 * BL], F32R, tag="uT")
                    nc.vector.tensor_copy(out=uhl[:, 0:BL], in_=tp_u[:])
                    nc.vector.tensor_tensor(out=uhl[:, BL:2 * BL], in0=tp_u[:],
                                            in1=uhl[:, 0:BL],
                                            op=mybir.AluOpType.subtract)
                else:
                    uT = actp.tile([128, BL], F32, tag="uT")
                    nc.vector.tensor_copy(out=uT[:], in_=tp_u[:])

                # L0: z0[64, 512] = u @ W0T
                if split:
                    p0 = ps_s.tile([2 * BL, W], F32, tag="pp")
                    nc.tensor.matmul(p0[:], lhsT=uhl[:], rhs=w0h[:],
                                     start=True, stop=False)
                    nc.tensor.matmul(p0[:], lhsT=uhl[:], rhs=w0l[:],
                                     start=False, stop=True)
                    z0 = actp.tile([BL, W], F32, tag="z0")
                    nc.vector.tensor_tensor(out=z0[:], in0=p0[0:BL, :],
                                            in1=p0[BL:2 * BL, :],
                                            op=mybir.AluOpType.add)
                    z0_ap = z0[:]
                else:
                    p0 = ps_s.tile([BL, W], F32, tag="pp")
                    nc.tensor.matmul(p0[:], lhsT=uT[:], rhs=w0t[:],
                                     start=True, stop=True)
                    z0_ap = p0[:]
                # softplus(x) = ln(exp(x) + 1) — Softplus has no PWP table in
                # this compiler build; Exp and Ln share one table set.
                e1 = actp.tile([BL, W], F32, tag="e1")
                nc.scalar.activation(e1[:], z0_ap,
                                     func=mybir.ActivationFunctionType.Exp)
                h1b = actp.tile([BL, W], F32, tag="h1b")
                nc.scalar.activation(h1b[:], e1[:],
                                     func=mybir.ActivationFunctionType.Ln,
                                     bias=1.0)

                # h1f chunks [128, 4*64]
                tp1 = ps_t.tile([128, KC * BL], F32, tag="tp")
                for k in range(KC):
                    nc.tensor.transpose(tp1[:, k * BL:(k + 1) * BL],
                                        h1b[:, k * 128:(k + 1) * 128], id64[:])
                if split:
                    h1hl = actp.tile([128, KC * 2 * BL], F32R, tag="h1f")
                    tp1_v = tp1[:].rearrange("p (k b) -> p k b", b=BL)
                    h1_v = h1hl[:].rearrange("p (k t) -> p k t", t=2 * BL)
                    nc.vector.tensor_copy(out=h1_v[:, :, 0:BL], in_=tp1_v)
                    nc.vector.tensor_tensor(out=h1_v[:, :, BL:2 * BL], in0=tp1_v,
                                            in1=h1_v[:, :, 0:BL],
                                            op=mybir.AluOpType.subtract)
                else:
                    h1f = actp.tile([128, KC * BL], F32, tag="h1f")
                    nc.vector.tensor_copy(out=h1f[:], in_=tp1[:])

                # L1: z1[64, 512] = sum_k h1_k @ W1T_k
                if split:
                    p1 = ps_s.tile([2 * BL, W], F32, tag="pp")
                    for k in range(KC):
                        for wt, first in ((w1h, k == 0), (w1l, False)):
                            nc.tensor.matmul(
                                p1[:],
                                lhsT=h1hl[:, k * 2 * BL:(k + 1) * 2 * BL],
                                rhs=wt[:, k * W:(k + 1) * W],
                                start=first,
                                stop=(k == KC - 1 and wt is w1l))
                    z1 = actp.tile([BL, W], F32, tag="z1")
                    nc.vector.tensor_tensor(out=z1[:], in0=p1[0:BL, :],
                                            in1=p1[BL:2 * BL, :],
                                            op=mybir.AluOpType.add)
                    z1_ap = z1[:]
                else:
                    p1 = ps_s.tile([BL, W], F32, tag="pp")
                    for k in range(KC):
                        nc.tensor.matmul(p1[:],
                                         lhsT=h1f[:, k * BL:(k + 1) * BL],
                                         rhs=w1t[:, k * W:(k + 1) * W],
                                         start=(k == 0), stop=(k == KC - 1))
                    z1_ap = p1[:]
                e2 = actp.tile([BL, W], F32, tag="e2")
                nc.scalar.activation(e2[:], z1_ap,
                                     func=mybir.ActivationFunctionType.Exp)
                h2b = actp.tile([BL, W], F32, tag="h2b")
                nc.scalar.activation(h2b[:], e2[:],
                                     func=mybir.ActivationFunctionType.Ln,
                                     bias=1.0)

                # h2f chunks
                tp2 = ps_t.tile([128, KC * BL], F32, tag="tp")
                for k in range(KC):
                    nc.tensor.transpose(tp2[:, k * BL:(k + 1) * BL],
                                        h2b[:, k * 128:(k + 1) * 128], id64[:])
                if split:
                    h2hl = actp.tile([128, KC * 2 * BL], F32R, tag="h2f")
                    tp2_v = tp2[:].rearrange("p (k b) -> p k b", b=BL)
                    h2_v = h2hl[:].rearrange("p (k t) -> p k t", t=2 * BL)
                    nc.vector.tensor_copy(out=h2_v[:, :, 0:BL], in_=tp2_v)
                    nc.vector.tensor_tensor(out=h2_v[:, :, BL:2 * BL], in0=tp2_v,
                                            in1=h2_v[:, :, 0:BL],
                                            op=mybir.AluOpType.subtract)
                else:
                    h2f = actp.tile([128, KC * BL], F32, tag="h2f")
                    nc.vector.tensor_copy(out=h2f[:], in_=tp2[:])

                # L2: z[64, 2048] = sum_k h2_k @ W2T_k ; n-chunks of 512
                g = actp.tile([BL, ND], F32, tag="g")
                if split:
                    pz = ps_l2.tile([2 * BL, ND], F32, tag="pz")
                    z2 = actp.tile([BL, ND], F32, tag="z2")
                    for n in range(NcN):
                        nsl = slice(n * 512, (n + 1) * 512)
                        for k in range(KC):
                            for wt, first in ((w2h, k == 0), (w2l, False)):
                                nc.tensor.matmul(
                                    pz[:, nsl],
                                    lhsT=h2hl[:, k * 2 * BL:(k + 1) * 2 * BL],
                                    rhs=wt[:, k * ND + n * 512:k * ND + (n + 1) * 512],
                                    start=first,
                                    stop=(k == KC - 1 and wt is w2l))
                        nc.vector.tensor_tensor(out=z2[:, nsl], in0=pz[0:BL, nsl],
                                                in1=pz[BL:2 * BL, nsl],
                                                op=mybir.AluOpType.add)
                        nc.scalar.activation(g[:, nsl], z2[:, nsl],
                                             func=mybir.ActivationFunctionType.Tanh)
                else:
                    pz = ps_l2.tile([BL, ND], F32, tag="pz")
                    for n in range(NcN):
                        nsl = slice(n * 512, (n + 1) * 512)
                        for k in range(KC):
                            nc.tensor.matmul(
                                pz[:, nsl],
                                lhsT=h2f[:, k * BL:(k + 1) * BL],
                                rhs=w2t[:, k * ND + n * 512:k * ND + (n + 1) * 512],
                                start=(k == 0), stop=(k == KC - 1))
                        nc.scalar.activation(g[:, nsl], pz[:, nsl],
                                             func=mybir.ActivationFunctionType.Tanh)

                # einsum: uadd[b,h] = sum_d g[b, h*16+d] * dX[b, s*16+d]
                ein = actp.tile([BL, ND], F32, tag="ein")
                dx_bc = dxs[:, s * D:(s + 1) * D].unsqueeze(1).broadcast_to([BL, H, D])
                nc.vector.tensor_tensor(
                    out=ein[:].rearrange("b (h d) -> b h d", d=D),
                    in0=g[:].rearrange("b (h d) -> b h d", d=D),
                    in1=dx_bc, op=mybir.AluOpType.mult)
                uadd = actp.tile([BL, H], F32, tag="uadd")
                nc.vector.tensor_reduce(
                    out=uadd[:], in_=ein[:].rearrange("b (h d) -> b h d", d=D),
                    axis=mybir.AxisListType.X, op=mybir.AluOpType.add)

                # lorenz drift + state update (replicates reference association)
                t1 = actp.tile([BL, H], F32, tag="t1")
                nc.vector.tensor_tensor(out=t1[:], in0=up1_ap, in1=um2_ap,
                                        op=mybir.AluOpType.subtract)
                t2 = actp.tile([BL, H], F32, tag="t2")
                nc.vector.tensor_tensor(out=t2[:], in0=t1[:], in1=um1_ap,
                                        op=mybir.AluOpType.mult)
                # lor = t2 - u + F ; u = (u + lor*DT) + uadd
                t3 = actp.tile([BL, H], F32, tag="t3")
                nc.vector.scalar_tensor_tensor(out=t3[:], in0=u_ap, scalar=-1.0,
                                               in1=t2[:], op0=mybir.AluOpType.mult,
                                               op1=mybir.AluOpType.add)  # t2 - u
                t4 = actp.tile([BL, H], F32, tag="t4")
                nc.vector.tensor_scalar(out=t4[:], in0=t3[:],
                                        scalar1=float(F_LORENZ), scalar2=float(DT),
                                        op0=mybir.AluOpType.add,
                                        op1=mybir.AluOpType.mult)  # (lor)*DT
                t5 = actp.tile([BL, H], F32, tag="t5")
                nc.vector.tensor_tensor(out=t5[:], in0=u_ap, in1=t4[:],
                                        op=mybir.AluOpType.add)  # u + lor*DT
                nc.vector.tensor_tensor(out=u_ap, in0=t5[:], in1=uadd[:],
                                        op=mybir.AluOpType.add)
                # refresh wrap columns
                nc.vector.tensor_copy(out=uext[:, 0:2], in_=uext[:, H:H + 2])
                nc.vector.tensor_copy(out=uext[:, H + 2:H + 4], in_=uext[:, 2:4])

            nc.sync.dma_start(out=out_d[:], in_=u_ap)

    nc.compile()
    return nc


_BUILT = {}


def _get_nc(stream_mode):
    if stream_mode not in _BUILT:
        _BUILT[stream_mode] = _build(stream_mode)
    return _BUILT[stream_mode]


class _Runner:
    """Caches the jitted shard_map executable and device-resident inputs so
    repeat kernel() calls pay only changed-input upload + device exec."""

    def __init__(self, nc):
        import jax
        import concourse.mybir as mybir
        from concourse import bass2jax
        from jax.sharding import Mesh, PartitionSpec
        from jax.experimental.shard_map import shard_map

        bass2jax.install_neuronx_cc_hook()
        self.jax = jax
        part_name = (nc.partition_id_tensor.name
                     if nc.partition_id_tensor is not None else None)
        in_names, out_names, out_avals = [], [], []
        for alloc in nc.m.functions[0].allocations:
            if not isinstance(alloc, mybir.MemoryLocationSet):
                continue
            name = alloc.memorylocations[0].name
            if alloc.kind == "ExternalInput":
                if name != part_name:
                    in_names.append(name)
            elif alloc.kind == "ExternalOutput":
                out_names.append(name)
                out_avals.append(jax.core.ShapedArray(
                    tuple(alloc.tensor_shape), mybir.dt.np(alloc.dtype)))
        self.in_names, self.out_names, self.out_avals = in_names, out_names, out_avals
        n_params, n_outs = len(in_names), len(out_names)
        all_names = in_names + out_names + ([part_name] if part_name else [])

        def _body(*args):
            operands = list(args)
            if part_name is not None:
                operands.append(bass2jax.partition_id_tensor())
            outs = bass2jax._bass_exec_p.bind(
                *operands,
                out_avals=tuple(out_avals),
                in_names=tuple(all_names),
                out_names=tuple(out_names),
                lowering_input_output_aliases=(),
                sim_require_finite=True,
                sim_require_nnan=True,
                nc=nc,
            )
            return tuple(outs)

        devices = jax.devices()[:NCORES]
        self.mesh = Mesh(np.asarray(devices), ("core",))
        in_specs = (PartitionSpec("core"),) * (n_params + n_outs)
        out_specs = (PartitionSpec("core"),) * n_outs
        self.fn = jax.jit(
            shard_map(_body, mesh=self.mesh, in_specs=in_specs,
                      out_specs=out_specs, check_rep=False),
            donate_argnums=tuple(range(n_params, n_params + n_outs)),
            keep_unused=True,
        )
        self.sharding = jax.sharding.NamedSharding(
            self.mesh, PartitionSpec("core"))
        self._dev_cache = {}   # name -> (bytes_digest, device_array)
        self.zero_outs = [np.zeros((NCORES * a.shape[0], *a.shape[1:]), a.dtype)
                          for a in out_avals]

    def _put(self, name, arr):
        import hashlib
        dig = hashlib.md5(arr.tobytes()).digest()
        hit = self._dev_cache.get(name)
        if hit is not None and hit[0] == dig:
            return hit[1]
        darr = self.jax.device_put(arr, self.sharding)
        darr.block_until_ready()
        self._dev_cache[name] = (dig, darr)
        return darr

    def run(self, concat_inputs: dict):
        args = [self._put(n, concat_inputs[n]) for n in self.in_names]
        zouts = [self.jax.device_put(z, self.sharding) for z in self.zero_outs]
        outs = self.fn(*args, *zouts)
        return {name: np.asarray(outs[i]) for i, name in enumerate(self.out_names)}


_RUNNERS = {}


def _get_runner(stream_mode):
    if stream_mode not in _RUNNERS:
        _RUNNERS[stream_mode] = _Runner(_get_nc(stream_mode))
    return _RUNNERS[stream_mode]


def kernel(u0, ts, coeff_a, coeff_b, coeff_c, coeff_d, W0, b0, W1, b1, W2, b2):
    u0 = np.ascontiguousarray(np.asarray(u0, np.float32))
    ts = np.asarray(ts, np.float32)
    coeff_a = np.asarray(coeff_a, np.float32)
    coeff_b = np.asarray(coeff_b, np.float32)
    coeff_c = np.asarray(coeff_c, np.float32)
    coeff_d = np.asarray(coeff_d, np.float32)
    assert not (np.any(b0) or np.any(b1) or np.any(b2)), \
        "nonzero MLP biases not supported by this kernel build"

    dX = _compute_dX(ts, coeff_a, coeff_b, coeff_c, coeff_d)  # [B, S, D]
    dXf = np.ascontiguousarray(dX.reshape(B, STEPS * D))

    w0t = np.ascontiguousarray(np.asarray(W0, np.float32).T)   # [128, 512]
    w1t = np.ascontiguousarray(np.asarray(W1, np.float32).T)   # [512, 512]
    w2t = np.ascontiguousarray(np.asarray(W2, np.float32).T)   # [512, 2048]
    ident = np.eye(BL, dtype=np.float32)

    # concat along axis 0: core c's slice is rows [c*dim0 : (c+1)*dim0]
    concat = dict(
        u0c=u0,                                   # already [8*64, H]
        dxc=dXf,                                  # [8*64, S*D]
        w0t=np.concatenate([w0t] * NCORES, 0),
        w1t=np.concatenate([w1t] * NCORES, 0),
        w2t=np.concatenate([w2t] * NCORES, 0),
        id64=np.concatenate([ident] * NCORES, 0),
    )

    runner = _get_runner(STREAM_MODE)
    outs = runner.run(concat)
    return np.ascontiguousarray(outs["uout"])  # [8*64, H] == [B, H]
